# revision 27
# baseline (speedup 1.0000x reference)
"""AdaptiveGridMerger Trainium2 kernel (v2: batch-pair array packing +
T-phase streaming pipeline).

Math: reference scatters x[b,c,:] into a flat 8x8 grid with bilinear
(4-corner) weights from positions[b,c,:], then matmuls grid_weights
GW [270,64].  Equivalent form, per batch:
  S in R[64,306]: column c = wy (x) wx (bilinear hat functions)
  gv  = S @ x          (mm1)
  out = GW @ gv        (mm2)

Sharding: data-parallel over batch, 2 batches per core, grid_weights
replicated (host pre-builds gw2 [128, 320] bf16 = GW.T stacked twice on
the partition axis, tail m-columns zero-padded 270->320).

Perf structure (per core, both batches processed together):
 - mm1 packs b0/b1 as two 128x64 column tiles of the PE array: b0's
   gv lands in PSUM partitions 0-63, b1's in 64-127, one 512-col
   stream window feeds both concurrently.
 - mm2 runs in 64x64 4-tile mode: contraction G=64, tiles
   (0,0)/(0,64)/(64,0)/(64,64) compute b0/b1 x m-lo/m-hi at once.
 - T is processed in 4 phases of 1024 cols; x input DMAs (sync/HWDGE)
   stream per phase so compute starts ~3.5us in, and output DMAs
   (gpsimd/SWDGE, separate queue) stream out results per phase while
   later input is still arriving -> DMA stays continuously busy near
   the HBM roofline.
 - PSUM->SBUF copies split across DVE and ACT; hat weights built on
   DVE (no activation table work on ACT).
"""

import numpy as np

import concourse.bass as bass
import concourse.bacc as bacc
import concourse.mybir as mybir
from concourse import tile
from concourse.bass_utils import run_bass_kernel_spmd

B, C, T = 16, 306, 4096
M, G, GS = 270, 64, 8
N_CORES = 8
BL = B // N_CORES  # batches per core

NPH = 4            # T phases
TPH = T // NPH     # 1024 cols per phase
TB = 512           # psum window cols
NTB = TPH // TB    # 2 windows per phase

CI = [(0, 128), (128, 128), (256, 50)]   # contraction chunks of C
MW = [(0, 128), (128, 128), (256, 64)]   # m windows (tail padded 14->64)
GW_COLS = 320                             # 270 padded to 64-multiple-ish

N_SPIN = 10

MM_DTYPE = mybir.dt.bfloat16
NP_MM = mybir.dt.np(MM_DTYPE)
FP32 = mybir.dt.float32
OP = mybir.AluOpType


def build_nc():
    nc = bacc.Bacc()
    x_ext = nc.declare_dram_parameter("x", [BL, C, T], MM_DTYPE, isOutput=False)
    pos_ext = nc.declare_dram_parameter("posp", [128, 12], FP32, isOutput=False)
    gw_ext = nc.declare_dram_parameter("gw2", [128, GW_COLS], MM_DTYPE, isOutput=False)
    out_ext = nc.declare_dram_parameter("out", [BL, M, T], MM_DTYPE, isOutput=True)

    with tile.TileContext(nc) as tc:
        with (
            tc.tile_pool(name="const", bufs=1) as constp,
            tc.tile_pool(name="xp", bufs=1) as xp,
            tc.tile_pool(name="op", bufs=1) as outp,
            tc.tile_pool(name="gvsb", bufs=3) as gvsbp,
            tc.tile_pool(name="gvps", bufs=2, space=bass.MemorySpace.PSUM) as gvpsp,
            tc.tile_pool(name="ps2", bufs=6, space=bass.MemorySpace.PSUM) as ps2p,
        ):
            # ---- PE pre-ramp spins (128x64 mode, same as mm1 ci0)
            dummy = constp.tile([128, TB], MM_DTYPE, tag="dummy")
            nc.vector.memset(dummy[:], 0.0)
            spin_ps = ps2p.tile([128, TB], FP32, tag="pb", name="spin_ps")
            for _ in range(N_SPIN):
                nc.tensor.matmul(
                    spin_ps[0:64, :], dummy[:, :64], dummy[:],
                    start=True, stop=True, skip_group_check=True,
                )

            # ---- input DMAs (sync / HWDGE): pos, gw, then x per phase
            # pos layout cols: (b, ci<2): 4b+2ci+d ; ci2: 8+2b+d
            pos_all = constp.tile([128, 12], FP32, tag="pos_all")
            nc.sync.dma_start(out=pos_all[:], in_=pos_ext[:])
            # x tiles: groups (ph0: 1024 cols, ph1: 1024, ph2+3: 2048).
            # xA [128, (ci2)(b2)(tw)] and xB [50, (b2)(tw)] per group.
            gw2 = constp.tile([128, GW_COLS], MM_DTYPE, tag="gw2")
            XGRP = [(ph * TPH, TPH) for ph in range(NPH)]
            xAg = {}
            xBg = {}
            # split input across both HWDGE rings: ci0 + xB on sync,
            # ci1 on scalar -- two rings stream concurrently.
            for gi, (t0, tw) in enumerate(XGRP):
                xa = xp.tile([128, 2 * BL * tw], MM_DTYPE, tag=f"xA{gi}", name=f"xA{gi}")
                if gi == 2:
                    nc.scalar.dma_start(out=gw2[:], in_=gw_ext[:])
                for ci, eng in ((0, nc.sync), (1, nc.scalar)):
                    eng.dma_start(
                        out=xa[:].rearrange("p (ci b t) -> p ci b t", ci=2, b=BL)[
                            :, ci, :, :
                        ],
                        in_=x_ext[:, ci * 128 : (ci + 1) * 128, t0 : t0 + tw].rearrange(
                            "b p t -> p b t"
                        ),
                    )
                xb = xp.tile([50, BL * tw], MM_DTYPE, tag=f"xB{gi}", name=f"xB{gi}")
                nc.sync.dma_start(
                    out=xb[:].rearrange("p (b t) -> p b t", b=BL),
                    in_=x_ext[:, 256:306, t0 : t0 + tw].rearrange("b p t -> p b t"),
                )
                xAg[gi] = xa
                xBg[gi] = xb

            def x_views(ph):
                xav = xAg[ph][:].rearrange("p (ci b t) -> p ci b t", ci=2, b=BL)
                xbv = xBg[ph][:].rearrange("p (b t) -> p b t", b=BL)
                return xav, xbv, 0

            # ---- iota row [0..7]
            io_g = constp.tile([128, GS], FP32, tag="io_g")
            nc.gpsimd.iota(
                io_g[:],
                pattern=[[1, GS]],
                base=0,
                channel_multiplier=0,
                allow_small_or_imprecise_dtypes=True,
            )
            io = constp.tile([128, GS], FP32, tag="io")
            nc.vector.tensor_copy(io[:], io_g[:])

            # ---- hat weights on DVE: w = max(0, min(1-(io-gp), 1+(io-gp)))
            gp = constp.tile([128, 12], FP32, tag="gp")
            nc.vector.tensor_scalar(gp[:], pos_all[:], 1.0, GS / 2.0, OP.add, OP.mult)
            d3 = constp.tile([128, 96], FP32, tag="d3")
            d3v = d3[:].rearrange("p (k j) -> p k j", k=12)
            nc.vector.tensor_tensor(
                d3v,
                io[:].unsqueeze(1).broadcast_to((128, 12, GS)),
                gp[:].unsqueeze(2).broadcast_to((128, 12, GS)),
                OP.subtract,
            )
            m1 = constp.tile([128, 96], FP32, tag="m1")
            nc.vector.tensor_scalar(m1[:], d3[:], -1.0, 1.0, OP.mult, OP.add)
            m2 = constp.tile([128, 96], FP32, tag="m2")
            nc.vector.tensor_scalar(m2[:], d3[:], 1.0, None, OP.add)
            mn3 = constp.tile([128, 96], FP32, tag="mn3")
            nc.vector.tensor_tensor(mn3[:], m1[:], m2[:], OP.min)
            w_all = constp.tile([128, 96], FP32, tag="w_all")
            nc.vector.tensor_scalar(w_all[:], mn3[:], 0.0, None, OP.max)

            # ---- st build: st[(b,ci)][c, 64] = wy (x) wx  (6 outer products)
            st_all = constp.tile([128, 6 * G], MM_DTYPE, tag="st_all")
            wv = w_all[:].rearrange("p (k j) -> p k j", k=12)

            def pos_col(b, ci, d):
                return (4 * b + 2 * ci + d) if ci < 2 else (8 + 2 * b + d)

            for ci in range(3):
                for b in range(BL):
                    k = b * 3 + ci
                    wy = wv[:, pos_col(b, ci, 0), :]
                    wx = wv[:, pos_col(b, ci, 1), :]
                    nc.vector.tensor_tensor(
                        st_all[:, k * G : (k + 1) * G].rearrange(
                            "p (i j) -> p i j", i=GS
                        ),
                        wy.unsqueeze(2).broadcast_to((128, GS, GS)),
                        wx.unsqueeze(1).broadcast_to((128, GS, GS)),
                        OP.mult,
                    )

            def st_sl(b, ci):
                k = b * 3 + ci
                cn = CI[ci][1]
                return st_all[:cn, k * G : (k + 1) * G]

            # ---- persistent output staging tiles
            outch = {}
            for b in range(BL):
                for h in range(2):
                    outch[(b, h)] = outp.tile(
                        [128, 4 * TPH], MM_DTYPE, tag=f"oc{b}_{h}", name=f"oc{b}_{h}"
                    )
            # tail staging: b0 rows at partitions 0-13, b1 rows at 64-77
            stage_pair = outp.tile([128, T], MM_DTYPE, tag="stgp", name="stgp")

            # ---- main pipeline: 8 steps of 512 cols, mm2 lags mm1 by one
            # step so every PSUM->SBUF copy has a full step of runway.
            k_copy = [0]

            def copy_any(dst, src):
                if k_copy[0] % 2 == 0:
                    nc.vector.tensor_copy(dst, src)
                else:
                    nc.scalar.copy(dst, src)
                k_copy[0] += 1

            NSTEP = T // TB  # 8
            N_FILL = {1: 4, 2: 4, 4: 4, 6: 4}
            gv_sbs = {}

            def emit_mm1(k):
                ph = k // NTB
                tb = k % NTB
                xav, xbv, toff = x_views(ph)
                ts = toff + tb * TB
                gv_ps = gvpsp.tile([128, TB], FP32, tag="gv", name=f"gv{k}")
                # HAM keep-warm filler: runs while waiting for x DMA, result
                # overwritten by the ci0 start=True matmul below.
                for _ in range(N_FILL.get(k, 0)):
                    nc.tensor.matmul(
                        gv_ps[0:64, :], dummy[:, :64], dummy[:],
                        start=True, stop=True, skip_group_check=True,
                    )
                for ci in range(3):
                    cn = CI[ci][1]
                    for b in range(BL):
                        if ci < 2:
                            rhs = xav[:cn, ci, b, ts : ts + TB]
                        else:
                            rhs = xbv[:cn, b, ts : ts + TB]
                        nc.tensor.matmul(
                            gv_ps[b * 64 : b * 64 + 64, :],
                            st_sl(b, ci),
                            rhs,
                            start=(ci == 0),
                            stop=(ci == 2),
                            skip_group_check=True,
                        )
                gv_sb = gvsbp.tile([128, TB], MM_DTYPE, tag="gvsb", name=f"gvsb{k}")
                copy_any(gv_sb[:], gv_ps[:])
                gv_sbs[k] = gv_sb

            def emit_mm2(k):
                ph = k // NTB
                tb = k % NTB
                t0 = ph * TPH
                ts = tb * TB
                gv_sb = gv_sbs[k]
                # tail pair tile: b0 -> parts 0-63 (tile 0,0), b1 -> parts
                # 64-127 (tile 64,64); the two MMs are emitted at opposite
                # ends of the step so they never touch the bank concurrently.
                o_t = ps2p.tile([128, TB], FP32, tag="pb", name=f"oT{k}")
                m0t, mnt = MW[2]
                nc.tensor.matmul(
                    o_t[0:64, :],
                    gw2[0:64, m0t : m0t + 64],
                    gv_sb[0:64, :],
                    start=True,
                    stop=True,
                    skip_group_check=True,
                )
                for mw, (m0, mn) in enumerate(MW[:2]):
                    o_a = ps2p.tile([128, TB], FP32, tag="pb", name=f"oA{k}_{mw}")
                    o_b = ps2p.tile([128, TB], FP32, tag="pb", name=f"oB{k}_{mw}")
                    for b, o_ps in ((0, o_a), (1, o_b)):
                        rhs = gv_sb[b * 64 : b * 64 + 64, :]
                        for h in range(mn // 64):
                            nc.tensor.matmul(
                                o_ps[h * 64 : h * 64 + 64, :],
                                gw2[b * 64 : b * 64 + 64, m0 + h * 64 : m0 + (h + 1) * 64],
                                rhs,
                                start=True,
                                stop=True,
                                skip_group_check=True,
                            )
                    h = ph // 2
                    hoff = mw * 2 * TPH + (ph % 2) * TPH + ts
                    for b, o_ps in ((0, o_a), (1, o_b)):
                        copy_any(
                            outch[(b, h)][:, hoff : hoff + TB],
                            o_ps[:],
                        )
                nc.tensor.matmul(
                    o_t[64:128, :],
                    gw2[64:128, m0t : m0t + 64],
                    gv_sb[64:128, :],
                    start=True,
                    stop=True,
                    skip_group_check=True,
                )
                copy_any(stage_pair[:, t0 + ts : t0 + ts + TB], o_t[:])

            def emit_dmas(k):
                ph = k // NTB
                tb = k % NTB
                t0 = ph * TPH
                ts = tb * TB
                if ph == NPH - 1:
                    # last phase: per-step chunks, issued on two queues in parallel
                    for b in range(BL):
                        eng = nc.sync if b == 0 else nc.scalar
                        eng.dma_start(
                            out=out_ext[b, 0:256, t0 + ts : t0 + ts + TB].rearrange(
                                "(mi p) t -> p mi t", p=128
                            ),
                            in_=outch[(b, 1)][:].rearrange(
                                "p (mi t) -> p mi t", mi=2
                            )[:, :, TPH + ts : TPH + ts + TB],
                        )
                elif tb == NTB - 1:
                    for b in range(BL):
                        nc.sync.dma_start(
                            out=out_ext[b, 0:256, t0 : t0 + TPH].rearrange(
                                "(mi p) t -> p mi t", p=128
                            ),
                            in_=outch[(b, ph // 2)][:].rearrange(
                                "p (mi t) -> p mi t", mi=2
                            )[:, :, (ph % 2) * TPH : (ph % 2) * TPH + TPH],
                        )
                if tb == NTB - 1:
                    for b in range(BL):
                        nc.sync.dma_start(
                            out=out_ext[b, 256:270, t0 : t0 + TPH],
                            in_=stage_pair[64 * b : 64 * b + 14, t0 : t0 + TPH],
                        )

            emit_mm1(0)
            for k in range(1, NSTEP):
                emit_mm1(k)
                emit_mm2(k - 1)
                emit_dmas(k - 1)
            emit_mm2(NSTEP - 1)
            emit_dmas(NSTEP - 1)
    nc.compile()
    return nc


def make_in_maps(x, positions, grid_weights):
    gw_t = np.ascontiguousarray(grid_weights.T).astype(NP_MM)  # [64, 270]
    gw2 = np.zeros((128, GW_COLS), dtype=NP_MM)
    gw2[0:64, 0:M] = gw_t
    gw2[64:128, 0:M] = gw_t
    in_maps = []
    for i in range(N_CORES):
        sl = slice(i * BL, (i + 1) * BL)
        ps = positions[sl]  # [BL, C, 2]
        posp = np.zeros((128, 12), dtype=np.float32)
        for b in range(BL):
            for ci in range(2):
                posp[:, 4 * b + 2 * ci : 4 * b + 2 * ci + 2] = ps[
                    b, ci * 128 : (ci + 1) * 128, :
                ]
            posp[:50, 8 + 2 * b : 10 + 2 * b] = ps[b, 256:306, :]
        in_maps.append(
            {
                "x": np.ascontiguousarray(x[sl]).astype(NP_MM),
                "posp": posp,
                "gw2": gw2,
            }
        )
    return in_maps


_NC_CACHE = None


def kernel(x, positions, grid_weights):
    global _NC_CACHE
    if _NC_CACHE is None:
        _NC_CACHE = build_nc()
    nc = _NC_CACHE
    in_maps = make_in_maps(x, positions, grid_weights)
    res = run_bass_kernel_spmd(nc, in_maps, core_ids=list(range(N_CORES)))
    out = np.concatenate([r["out"] for r in res.results], axis=0)
    return np.asarray(out, dtype=np.float32)


if __name__ == "__main__":
    xs = np.random.randn(B, C, T).astype(np.float32)
    ps = np.random.uniform(-1, 0.74, (B, C, 2)).astype(np.float32)
    gw = np.random.randn(M, G).astype(np.float32)
    out = kernel(xs, ps, gw)
    print(out.shape, out.dtype)
